# revision 16
# baseline (speedup 1.0000x reference)
"""Sparse paged-attention decode kernel for 8 TRN2 NeuronCores.

Strategy v3 (batch-parallel, fat-row gathers, batched softmax):
  - Requests sorted by context length; core i owns requests (order[i],
    order[15-i]) - a long+short pair - all 8 KV heads, their 32 q heads.
  - Host builds a per-core compact pool of referenced KV-cache rows
    (np.unique remap -> int16 indices), applies the slot_mapping scatter,
    stores rows as 8-head stripes kpool/vpool [npool, 8*128] bf16 (2KB).
  - Device: dma_gather(transpose=True, elem=1024) pulls K rows as per-head
    K^T tiles (d on partitions - zero on-chip transposes); plain
    dma_gather(elem=1024) pulls V in natural [slot, d] layout. Fat 2KB rows
    keep SWDGE descriptor generation (the real paged-attention bottleneck)
    8x cheaper than per-head rows. A tiny warm-up gather triggers the Q7
    ucode IRAM load while input DMAs run.
  - Compute per (request r, head h), head-major so PSUM accumulation groups
    never interleave within a bank (start=True clears the whole bank's
    has_written bits):
      for each 128-slot chunk: S^T = matmul(K^T chunk, Q^T) -> PSUM [128, nch*4]
      one ACT exp over the whole group                      -> SBUF bf16
      one DVE multiply by 0/1 mask (pads/dummies -> 0)
      for each chunk: O += matmul(P^T, V chunk); den += matmul(P^T, ones)
    Epilogue: out = O * reciprocal(den); single DMA out.
  - Softmax skips max-subtraction (|scores| < ~8 for N(0,1) q/k).
"""

import sys

if "/opt/trn_rl_repo" not in sys.path:
    sys.path.insert(0, "/opt/trn_rl_repo")

from contextlib import ExitStack

import ml_dtypes
import numpy as np

import concourse.mybir as mybir
from concourse import bacc, bass_utils, tile

BF16 = ml_dtypes.bfloat16

B = 16
H = 32
HKV = 8
G = H // HKV
D = 128
MAX_CTX = 2048
NUM_SLOTS = B * MAX_CTX + B
SCALE = 1.0 / np.sqrt(D)
NCORES = 8
RPC = 2
CH = 128
PERK = 128   # idx per K transpose-gather (8 descriptors/idx)
PERV = 256   # idx per V gather
ROW = HKV * D

_compiled = {}


def _build(npool, nch_r, idx_cols):
    nc = bacc.Bacc(
        "TRN2", target_bir_lowering=False, debug=False, num_swdge_queues=2
    )
    dt = mybir.dt
    nch = sum(nch_r)
    n_idx = nch * CH
    cum = [0, nch_r[0]]

    kpool = nc.dram_tensor("kpool", [npool, ROW], dt.bfloat16, kind="ExternalInput")
    vpool = nc.dram_tensor("vpool", [npool, ROW], dt.bfloat16, kind="ExternalInput")
    qt_d = nc.dram_tensor("qt", [D, RPC * H], dt.bfloat16, kind="ExternalInput")
    idx_d = nc.dram_tensor("idx", [128, idx_cols], dt.int16, kind="ExternalInput")
    mask_d = nc.dram_tensor("mask", [128, nch * G], dt.bfloat16, kind="ExternalInput")
    out_d = nc.dram_tensor("o", [G, RPC * HKV * D], dt.float32, kind="ExternalOutput")
    deno_d = nc.dram_tensor("deno", [G, RPC * HKV], dt.float32, kind="ExternalOutput")

    with tile.TileContext(nc) as tc:
        with ExitStack() as ctx:
            const = ctx.enter_context(tc.tile_pool(name="const", bufs=1))
            stp = ctx.enter_context(tc.tile_pool(name="st", bufs=2, space="PSUM"))
            accp = ctx.enter_context(tc.tile_pool(name="acc", bufs=1, space="PSUM"))

            # warm-up gather: loads the Q7 SWDGE ucode IRAM (~6us) while the
            # real inputs stream in; gathers row 0 x16 into a scratch tile.
            warm_idx = const.tile([128, 1], dt.int16)
            nc.vector.memset(warm_idx[:], 0)
            warm_dst = const.tile([128, ROW], dt.bfloat16)
            nc.gpsimd.dma_gather(
                warm_dst[:].rearrange("p (b e) -> p b e", b=1),
                kpool.ap()[:, :],
                warm_idx[:],
                16,
                16,
                ROW,
                single_packet=False,
            )

            idx_t = const.tile([128, idx_cols], dt.int16)
            nc.sync.dma_start(idx_t[:], idx_d.ap()[:, :])
            qt_t = const.tile([D, RPC * H], dt.bfloat16)
            nc.sync.dma_start(qt_t[:], qt_d.ap()[:, :])
            mask_t = const.tile([128, nch * G], dt.bfloat16)
            nc.sync.dma_start(mask_t[:], mask_d.ap()[:, :])
            ones_t = const.tile([128, 1], dt.bfloat16)
            nc.vector.memset(ones_t[:], 1.0)

            kt_t = const.tile([128, HKV * n_idx], dt.bfloat16)
            v_t = const.tile([128, HKV * n_idx], dt.bfloat16)
            expp_t = const.tile([128, nch * HKV * G], dt.bfloat16)
            out_t = const.tile([G, RPC * HKV * D], dt.float32)
            rden_t = const.tile([G, RPC * HKV], dt.float32)

            o_acc = accp.tile([G, RPC * HKV * D], dt.float32)
            den = accp.tile([G, RPC * HKV], dt.float32)

            kg_sems = [
                nc.alloc_semaphore(f"kg{i}")
                for i in range((n_idx + PERK - 1) // PERK)
            ]
            vg_sems = [
                nc.alloc_semaphore(f"vg{i}")
                for i in range((n_idx + PERV - 1) // PERV)
            ]

            issue = []
            for g0 in range(0, n_idx, PERK):
                issue.append(("k", g0))
            for g0 in range(0, n_idx, PERV):
                issue.append(("v", g0))
            # interleave by data position so K and V of the same chunks
            # arrive together; alternate the two SWDGE queues
            issue.sort(key=lambda t: (t[1], t[0]))
            for kind, g0 in issue:
                if kind == "k":
                    ng = min(PERK, n_idx - g0)
                    nc.gpsimd.dma_gather(
                        kt_t[:, g0 * HKV : (g0 + ng) * HKV].rearrange(
                            "p (b e) -> p b e", b=HKV
                        ),
                        kpool.ap()[:, :],
                        idx_t[:, g0 // 16 : (g0 + ng) // 16],
                        ng,
                        ng,
                        ROW,
                        transpose=True,
                        single_packet=False,
                        queue_num=0,
                    ).then_inc(kg_sems[g0 // PERK], 16)
                else:
                    ng = min(PERV, n_idx - g0)
                    nc.gpsimd.dma_gather(
                        v_t[:, g0 * HKV : (g0 + ng) * HKV].rearrange(
                            "p (b e) -> p b e", e=ROW
                        ),
                        vpool.ap()[:, :],
                        idx_t[:, g0 // 16 : (g0 + ng) // 16],
                        ng,
                        ng,
                        ROW,
                        single_packet=False,
                        queue_num=1,
                    ).then_inc(vg_sems[g0 // PERV], 16)

            def kt_slice(c, h):
                g, loc = divmod(c * CH, PERK)
                base = g * HKV * PERK + h * PERK + loc
                return kt_t[:, base : base + CH]

            # head-major: each (r, h) PSUM accumulation group completes
            # before the next starts (start=True clears the whole bank's
            # has_written bits, so groups sharing a bank must not interleave)
            for r in range(RPC):
                nch_l = nch_r[r]
                c0 = cum[r]
                for h in range(HKV):
                    blk = r * HKV + h
                    st = stp.tile([128, nch_r[0] * G], dt.float32, tag="st")
                    base = (c0 * HKV + h * nch_l) * G
                    for cl in range(nch_l):
                        nc.tensor.matmul(
                            st[:, cl * G : (cl + 1) * G],
                            kt_slice(c0 + cl, h),
                            qt_t[:, blk * G : (blk + 1) * G],
                            start=True,
                            stop=True,
                        )._wait_ge(kg_sems[(c0 + cl) * CH // PERK], 16)
                    pt = expp_t[:, base : base + nch_l * G]
                    nc.scalar.activation(
                        pt,
                        st[:, 0 : nch_l * G],
                        mybir.ActivationFunctionType.Exp,
                    )
                    nc.vector.tensor_mul(
                        pt, pt, mask_t[:, c0 * G : (c0 + nch_l) * G]
                    )
                    for cl in range(nch_l):
                        c = c0 + cl
                        ptc = expp_t[:, base + cl * G : base + (cl + 1) * G]
                        nc.tensor.matmul(
                            o_acc[:, blk * D : (blk + 1) * D],
                            ptc,
                            v_t[:, (c * HKV + h) * D : (c * HKV + h + 1) * D],
                            start=(cl == 0),
                            stop=(cl == nch_l - 1),
                            skip_group_check=True,
                        )._wait_ge(vg_sems[c * CH // PERV], 16)
                        nc.tensor.matmul(
                            den[:, blk : blk + 1],
                            ptc,
                            ones_t[:],
                            start=(cl == 0),
                            stop=(cl == nch_l - 1),
                            skip_group_check=True,
                        )
            # numerator/denominator go to host; division (flash-decode
            # stage-2 combine) happens there on the tiny output tensor
            nc.scalar.copy(out_t[:], o_acc[:])
            nc.vector.tensor_copy(rden_t[:], den[:])
            nc.sync.dma_start(out_d.ap()[:, :], out_t[:])
            nc.sync.dma_start(deno_d.ap()[:, :], rden_t[:])

    nc.compile()
    return nc


def kernel(q, k, v, k_cache, v_cache, slot_mapping, active_slots, context_lens):
    q = np.asarray(q)
    k = np.asarray(k)
    v = np.asarray(v)
    k_cache = np.asarray(k_cache)
    v_cache = np.asarray(v_cache)
    slot_mapping = np.asarray(slot_mapping)
    active_slots = np.asarray(active_slots)
    context_lens = np.asarray(context_lens).astype(np.int64)

    order = np.argsort(-context_lens, kind="stable")
    pairs = [(int(order[i]), int(order[B - 1 - i])) for i in range(NCORES)]

    nch_r = tuple(
        max(int(-(-context_lens[p[s]] // CH)) for p in pairs) for s in range(RPC)
    )
    nch = sum(nch_r)
    n_idx = nch * CH

    kc_new = k.astype(BF16)
    vc_new = v.astype(BF16)
    sm_ok = {}
    for i in range(B):
        s = int(slot_mapping[i])
        if 0 <= s < NUM_SLOTS:
            sm_ok[s] = i

    per_core = []
    for core in range(NCORES):
        rA, rB = pairs[core]
        flat = np.zeros(n_idx, np.int64)
        mask = np.zeros((128, nch * G), BF16)
        for s, r in enumerate((rA, rB)):
            L = int(context_lens[r])
            off = 0 if s == 0 else nch_r[0]
            flat[off * CH : off * CH + L] = active_slots[r, :L]
            for c in range(nch_r[s]):
                nv = min(max(L - c * CH, 0), CH)
                if nv > 0:
                    mask[:nv, (off + c) * G : (off + c + 1) * G] = 1.0
        uniq, inv = np.unique(flat, return_inverse=True)
        npool = len(uniq)
        assert npool < 32768
        kp = k_cache[uniq].astype(BF16).reshape(npool, ROW)
        vp = v_cache[uniq].astype(BF16).reshape(npool, ROW)
        for pos, s in enumerate(uniq):
            i = sm_ok.get(int(s))
            if i is not None:
                kp[pos] = kc_new[i].reshape(ROW)
                vp[pos] = vc_new[i].reshape(ROW)
        idx16 = inv.astype(np.int16)
        idx_w = np.tile(idx16.reshape(n_idx // 16, 16).T, (8, 1))

        qs = (q[(rA, rB), :, :] * SCALE).astype(BF16)
        qt = np.ascontiguousarray(qs.transpose(2, 0, 1).reshape(D, RPC * H))
        per_core.append(
            {"kp": kp, "vp": vp, "qt": qt, "idx": idx_w, "mask": mask}
        )

    npool_max = max(pc["kp"].shape[0] for pc in per_core)
    in_maps = []
    for pc in per_core:
        kp, vp = pc["kp"], pc["vp"]
        if kp.shape[0] < npool_max:
            pad = np.zeros((npool_max - kp.shape[0], ROW), BF16)
            kp = np.concatenate([kp, pad])
            vp = np.concatenate([vp, pad])
        in_maps.append(
            {
                "kpool": kp,
                "vpool": vp,
                "qt": pc["qt"],
                "idx": pc["idx"],
                "mask": pc["mask"],
            }
        )

    idx_cols = n_idx // 16
    key = (npool_max, nch_r, idx_cols)
    if key not in _compiled:
        _compiled[key] = _build(npool_max, nch_r, idx_cols)
    nc = _compiled[key]

    res = bass_utils.run_bass_kernel_spmd(nc, in_maps, core_ids=list(range(NCORES)))

    out = np.empty((B, H, D), np.float32)
    for core in range(NCORES):
        num = res.results[core]["o"].reshape(G, RPC, HKV, D)
        dn = res.results[core]["deno"].reshape(G, RPC, HKV, 1)
        o = num / dn
        for s, r in enumerate(pairs[core]):
            out[r] = o[:, s, :, :].transpose(1, 0, 2).reshape(H, D)
    return out


# revision 17
# speedup vs baseline: 1.5184x; 1.5184x over previous
"""Sparse paged-attention decode kernel for 8 TRN2 NeuronCores.

Strategy v3 (batch-parallel, fat-row gathers, batched softmax):
  - Requests sorted by context length; core i owns requests (order[i],
    order[15-i]) - a long+short pair - all 8 KV heads, their 32 q heads.
  - Host builds a per-core compact pool of referenced KV-cache rows
    (np.unique remap -> int16 indices), applies the slot_mapping scatter,
    stores rows as 8-head stripes kpool/vpool [npool, 8*128] bf16 (2KB).
  - Device: dma_gather(transpose=True, elem=1024) pulls K rows as per-head
    K^T tiles (d on partitions - zero on-chip transposes); plain
    dma_gather(elem=1024) pulls V in natural [slot, d] layout. Fat 2KB rows
    keep SWDGE descriptor generation (the real paged-attention bottleneck)
    8x cheaper than per-head rows. A tiny warm-up gather triggers the Q7
    ucode IRAM load while input DMAs run.
  - Compute per (request r, head h), head-major so PSUM accumulation groups
    never interleave within a bank (start=True clears the whole bank's
    has_written bits):
      for each 128-slot chunk: S^T = matmul(K^T chunk, Q^T) -> PSUM [128, nch*4]
      one ACT exp over the whole group                      -> SBUF bf16
      one DVE multiply by 0/1 mask (pads/dummies -> 0)
      for each chunk: O += matmul(P^T, V chunk); den += matmul(P^T, ones)
    Epilogue: out = O * reciprocal(den); single DMA out.
  - Softmax skips max-subtraction (|scores| < ~8 for N(0,1) q/k).
"""

import sys

if "/opt/trn_rl_repo" not in sys.path:
    sys.path.insert(0, "/opt/trn_rl_repo")

from contextlib import ExitStack

import ml_dtypes
import numpy as np

import concourse.mybir as mybir
from concourse import bacc, bass_utils, tile

BF16 = ml_dtypes.bfloat16

B = 16
H = 32
HKV = 8
G = H // HKV
D = 128
MAX_CTX = 2048
NUM_SLOTS = B * MAX_CTX + B
SCALE = 1.0 / np.sqrt(D)
NCORES = 8
RPC = 2
CH = 128
PERK = 256   # idx per K transpose-gather (8 descriptors/idx)
PERV = 384   # idx per V gather
ROW = HKV * D

_compiled = {}


def _build(npool, nch_r, idx_cols):
    nc = bacc.Bacc(
        "TRN2", target_bir_lowering=False, debug=False, num_swdge_queues=2
    )
    dt = mybir.dt
    nch = sum(nch_r)
    n_idx = nch * CH
    cum = [0, nch_r[0]]

    kpool = nc.dram_tensor("kpool", [npool, ROW], dt.bfloat16, kind="ExternalInput")
    vpool = nc.dram_tensor("vpool", [npool, ROW], dt.bfloat16, kind="ExternalInput")
    qt_d = nc.dram_tensor("qt", [D, RPC * H], dt.bfloat16, kind="ExternalInput")
    idx_d = nc.dram_tensor("idx", [128, idx_cols], dt.int16, kind="ExternalInput")
    mask_d = nc.dram_tensor("mask", [128, nch * G], dt.bfloat16, kind="ExternalInput")
    out_d = nc.dram_tensor("o", [G, RPC * HKV * D], dt.float32, kind="ExternalOutput")
    deno_d = nc.dram_tensor("deno", [G, RPC * HKV], dt.float32, kind="ExternalOutput")

    with tile.TileContext(nc) as tc:
        with ExitStack() as ctx:
            const = ctx.enter_context(tc.tile_pool(name="const", bufs=1))
            stp = ctx.enter_context(tc.tile_pool(name="st", bufs=2, space="PSUM"))
            accp = ctx.enter_context(tc.tile_pool(name="acc", bufs=1, space="PSUM"))

            # warm-up gather: loads the Q7 SWDGE ucode IRAM (~6us) while the
            # real inputs stream in; gathers row 0 x16 into a scratch tile.
            warm_idx = const.tile([128, 1], dt.int16)
            nc.vector.memset(warm_idx[:], 0)
            warm_dst = const.tile([128, ROW], dt.bfloat16)
            nc.gpsimd.dma_gather(
                warm_dst[:].rearrange("p (b e) -> p b e", b=1),
                kpool.ap()[:, :],
                warm_idx[:],
                16,
                16,
                ROW,
                single_packet=False,
            )

            idx_t = const.tile([128, idx_cols], dt.int16)
            nc.sync.dma_start(idx_t[:], idx_d.ap()[:, :])
            qt_t = const.tile([D, RPC * H], dt.bfloat16)
            nc.sync.dma_start(qt_t[:], qt_d.ap()[:, :])
            mask_t = const.tile([128, nch * G], dt.bfloat16)
            nc.sync.dma_start(mask_t[:], mask_d.ap()[:, :])
            ones_t = const.tile([128, 1], dt.bfloat16)
            nc.vector.memset(ones_t[:], 1.0)

            kt_t = const.tile([128, HKV * n_idx], dt.bfloat16)
            v_t = const.tile([128, HKV * n_idx], dt.bfloat16)
            expp_t = const.tile([128, nch * HKV * G], dt.bfloat16)
            out_t = const.tile([G, RPC * HKV * D], dt.float32)
            rden_t = const.tile([G, RPC * HKV], dt.float32)

            o_acc = accp.tile([G, RPC * HKV * D], dt.float32)
            den = accp.tile([G, RPC * HKV], dt.float32)

            kg_sems = [
                nc.alloc_semaphore(f"kg{i}")
                for i in range((n_idx + PERK - 1) // PERK)
            ]
            vg_sems = [
                nc.alloc_semaphore(f"vg{i}")
                for i in range((n_idx + PERV - 1) // PERV)
            ]

            issue = []
            for g0 in range(0, n_idx, PERK):
                issue.append(("k", g0))
            for g0 in range(0, n_idx, PERV):
                issue.append(("v", g0))
            # interleave by data position so K and V of the same chunks
            # arrive together; alternate the two SWDGE queues
            issue.sort(key=lambda t: (t[1], t[0]))
            for kind, g0 in issue:
                if kind == "k":
                    ng = min(PERK, n_idx - g0)
                    nc.gpsimd.dma_gather(
                        kt_t[:, g0 * HKV : (g0 + ng) * HKV].rearrange(
                            "p (b e) -> p b e", b=HKV
                        ),
                        kpool.ap()[:, :],
                        idx_t[:, g0 // 16 : (g0 + ng) // 16],
                        ng,
                        ng,
                        ROW,
                        transpose=True,
                        single_packet=False,
                        queue_num=0,
                    ).then_inc(kg_sems[g0 // PERK], 16)
                else:
                    ng = min(PERV, n_idx - g0)
                    nc.gpsimd.dma_gather(
                        v_t[:, g0 * HKV : (g0 + ng) * HKV].rearrange(
                            "p (b e) -> p b e", e=ROW
                        ),
                        vpool.ap()[:, :],
                        idx_t[:, g0 // 16 : (g0 + ng) // 16],
                        ng,
                        ng,
                        ROW,
                        single_packet=False,
                        queue_num=1,
                    ).then_inc(vg_sems[g0 // PERV], 16)

            def kt_slice(c, h):
                g, loc = divmod(c * CH, PERK)
                base = g * HKV * PERK + h * PERK + loc
                return kt_t[:, base : base + CH]

            # head-major: each (r, h) PSUM accumulation group completes
            # before the next starts (start=True clears the whole bank's
            # has_written bits, so groups sharing a bank must not interleave)
            for r in range(RPC):
                nch_l = nch_r[r]
                c0 = cum[r]
                for h in range(HKV):
                    blk = r * HKV + h
                    st = stp.tile([128, nch_r[0] * G], dt.float32, tag="st")
                    base = (c0 * HKV + h * nch_l) * G
                    for cl in range(nch_l):
                        nc.tensor.matmul(
                            st[:, cl * G : (cl + 1) * G],
                            kt_slice(c0 + cl, h),
                            qt_t[:, blk * G : (blk + 1) * G],
                            start=True,
                            stop=True,
                        )._wait_ge(kg_sems[(c0 + cl) * CH // PERK], 16)
                    pt = expp_t[:, base : base + nch_l * G]
                    nc.scalar.activation(
                        pt,
                        st[:, 0 : nch_l * G],
                        mybir.ActivationFunctionType.Exp,
                    )
                    nc.vector.tensor_mul(
                        pt, pt, mask_t[:, c0 * G : (c0 + nch_l) * G]
                    )
                    for cl in range(nch_l):
                        c = c0 + cl
                        ptc = expp_t[:, base + cl * G : base + (cl + 1) * G]
                        nc.tensor.matmul(
                            o_acc[:, blk * D : (blk + 1) * D],
                            ptc,
                            v_t[:, (c * HKV + h) * D : (c * HKV + h + 1) * D],
                            start=(cl == 0),
                            stop=(cl == nch_l - 1),
                            skip_group_check=True,
                        )._wait_ge(vg_sems[c * CH // PERV], 16)
                        nc.tensor.matmul(
                            den[:, blk : blk + 1],
                            ptc,
                            ones_t[:],
                            start=(cl == 0),
                            stop=(cl == nch_l - 1),
                            skip_group_check=True,
                        )
            # numerator/denominator go to host; division (flash-decode
            # stage-2 combine) happens there on the tiny output tensor
            nc.scalar.copy(out_t[:], o_acc[:])
            nc.vector.tensor_copy(rden_t[:], den[:])
            nc.sync.dma_start(out_d.ap()[:, :], out_t[:])
            nc.sync.dma_start(deno_d.ap()[:, :], rden_t[:])

    nc.compile()
    return nc


def kernel(q, k, v, k_cache, v_cache, slot_mapping, active_slots, context_lens):
    q = np.asarray(q)
    k = np.asarray(k)
    v = np.asarray(v)
    k_cache = np.asarray(k_cache)
    v_cache = np.asarray(v_cache)
    slot_mapping = np.asarray(slot_mapping)
    active_slots = np.asarray(active_slots)
    context_lens = np.asarray(context_lens).astype(np.int64)

    order = np.argsort(-context_lens, kind="stable")
    pairs = [(int(order[i]), int(order[B - 1 - i])) for i in range(NCORES)]

    nch_r = tuple(
        max(int(-(-context_lens[p[s]] // CH)) for p in pairs) for s in range(RPC)
    )
    nch = sum(nch_r)
    n_idx = nch * CH

    kc_new = k.astype(BF16)
    vc_new = v.astype(BF16)
    sm_ok = {}
    for i in range(B):
        s = int(slot_mapping[i])
        if 0 <= s < NUM_SLOTS:
            sm_ok[s] = i

    per_core = []
    for core in range(NCORES):
        rA, rB = pairs[core]
        flat = np.zeros(n_idx, np.int64)
        mask = np.zeros((128, nch * G), BF16)
        for s, r in enumerate((rA, rB)):
            L = int(context_lens[r])
            off = 0 if s == 0 else nch_r[0]
            flat[off * CH : off * CH + L] = active_slots[r, :L]
            for c in range(nch_r[s]):
                nv = min(max(L - c * CH, 0), CH)
                if nv > 0:
                    mask[:nv, (off + c) * G : (off + c + 1) * G] = 1.0
        uniq, inv = np.unique(flat, return_inverse=True)
        npool = len(uniq)
        assert npool < 32768
        kp = k_cache[uniq].astype(BF16).reshape(npool, ROW)
        vp = v_cache[uniq].astype(BF16).reshape(npool, ROW)
        for pos, s in enumerate(uniq):
            i = sm_ok.get(int(s))
            if i is not None:
                kp[pos] = kc_new[i].reshape(ROW)
                vp[pos] = vc_new[i].reshape(ROW)
        idx16 = inv.astype(np.int16)
        idx_w = np.tile(idx16.reshape(n_idx // 16, 16).T, (8, 1))

        qs = (q[(rA, rB), :, :] * SCALE).astype(BF16)
        qt = np.ascontiguousarray(qs.transpose(2, 0, 1).reshape(D, RPC * H))
        per_core.append(
            {"kp": kp, "vp": vp, "qt": qt, "idx": idx_w, "mask": mask}
        )

    npool_max = max(pc["kp"].shape[0] for pc in per_core)
    in_maps = []
    for pc in per_core:
        kp, vp = pc["kp"], pc["vp"]
        if kp.shape[0] < npool_max:
            pad = np.zeros((npool_max - kp.shape[0], ROW), BF16)
            kp = np.concatenate([kp, pad])
            vp = np.concatenate([vp, pad])
        in_maps.append(
            {
                "kpool": kp,
                "vpool": vp,
                "qt": pc["qt"],
                "idx": pc["idx"],
                "mask": pc["mask"],
            }
        )

    idx_cols = n_idx // 16
    key = (npool_max, nch_r, idx_cols)
    if key not in _compiled:
        _compiled[key] = _build(npool_max, nch_r, idx_cols)
    nc = _compiled[key]

    res = bass_utils.run_bass_kernel_spmd(nc, in_maps, core_ids=list(range(NCORES)))

    out = np.empty((B, H, D), np.float32)
    for core in range(NCORES):
        num = res.results[core]["o"].reshape(G, RPC, HKV, D)
        dn = res.results[core]["deno"].reshape(G, RPC, HKV, 1)
        o = num / dn
        for s, r in enumerate(pairs[core]):
            out[r] = o[:, s, :, :].transpose(1, 0, 2).reshape(H, D)
    return out


# revision 18
# speedup vs baseline: 1.6207x; 1.0673x over previous
"""Sparse paged-attention decode kernel for 8 TRN2 NeuronCores.

Strategy v3 (batch-parallel, fat-row gathers, batched softmax):
  - Requests sorted by context length; core i owns requests (order[i],
    order[15-i]) - a long+short pair - all 8 KV heads, their 32 q heads.
  - Host builds a per-core compact pool of referenced KV-cache rows
    (np.unique remap -> int16 indices), applies the slot_mapping scatter,
    stores rows as 8-head stripes kpool/vpool [npool, 8*128] bf16 (2KB).
  - Device: dma_gather(transpose=True, elem=1024) pulls K rows as per-head
    K^T tiles (d on partitions - zero on-chip transposes); plain
    dma_gather(elem=1024) pulls V in natural [slot, d] layout. Fat 2KB rows
    keep SWDGE descriptor generation (the real paged-attention bottleneck)
    8x cheaper than per-head rows. A tiny warm-up gather triggers the Q7
    ucode IRAM load while input DMAs run.
  - Compute per (request r, head h), head-major so PSUM accumulation groups
    never interleave within a bank (start=True clears the whole bank's
    has_written bits):
      for each 128-slot chunk: S^T = matmul(K^T chunk, Q^T) -> PSUM [128, nch*4]
      one ACT exp over the whole group                      -> SBUF bf16
      one DVE multiply by 0/1 mask (pads/dummies -> 0)
      for each chunk: O += matmul(P^T, V chunk); den += matmul(P^T, ones)
    Epilogue: out = O * reciprocal(den); single DMA out.
  - Softmax skips max-subtraction (|scores| < ~8 for N(0,1) q/k).
"""

import sys

if "/opt/trn_rl_repo" not in sys.path:
    sys.path.insert(0, "/opt/trn_rl_repo")

from contextlib import ExitStack

import ml_dtypes
import numpy as np

import concourse.mybir as mybir
from concourse import bacc, bass_utils, tile

BF16 = ml_dtypes.bfloat16

B = 16
H = 32
HKV = 8
G = H // HKV
D = 128
MAX_CTX = 2048
NUM_SLOTS = B * MAX_CTX + B
SCALE = 1.0 / np.sqrt(D)
NCORES = 8
RPC = 2
CH = 128
PERK = 256   # idx per K transpose-gather (8 descriptors/idx)
PERV = 384   # idx per V gather
ROW = HKV * D

_compiled = {}


def _build(npool, nch_r, idx_cols):
    nc = bacc.Bacc(
        "TRN2", target_bir_lowering=False, debug=False, num_swdge_queues=2
    )
    dt = mybir.dt
    nch = sum(nch_r)
    n_idx = nch * CH
    cum = [0, nch_r[0]]

    kpool = nc.dram_tensor("kpool", [npool, ROW], dt.bfloat16, kind="ExternalInput")
    vpool = nc.dram_tensor("vpool", [npool, ROW], dt.bfloat16, kind="ExternalInput")
    qt_d = nc.dram_tensor("qt", [D, RPC * H], dt.bfloat16, kind="ExternalInput")
    idx_d = nc.dram_tensor("idx", [128, idx_cols], dt.int16, kind="ExternalInput")
    mask_d = nc.dram_tensor("mask", [128, nch * G], dt.bfloat16, kind="ExternalInput")
    out_d = nc.dram_tensor("o", [G, RPC * HKV * D], dt.float32, kind="ExternalOutput")
    deno_d = nc.dram_tensor("deno", [G, RPC * HKV], dt.float32, kind="ExternalOutput")

    with tile.TileContext(nc) as tc:
        with ExitStack() as ctx:
            const = ctx.enter_context(tc.tile_pool(name="const", bufs=1))
            stp = ctx.enter_context(tc.tile_pool(name="st", bufs=2, space="PSUM"))
            accp = ctx.enter_context(tc.tile_pool(name="acc", bufs=1, space="PSUM"))

            # warm-up gather: loads the Q7 SWDGE ucode IRAM (~6us) while the
            # real inputs stream in; gathers row 0 x16 into a scratch tile.
            warm_idx = const.tile([128, 1], dt.int16)
            nc.vector.memset(warm_idx[:], 0)
            warm_dst = const.tile([128, ROW], dt.bfloat16)
            nc.gpsimd.dma_gather(
                warm_dst[:].rearrange("p (b e) -> p b e", b=1),
                kpool.ap()[:, :],
                warm_idx[:],
                16,
                16,
                ROW,
                single_packet=False,
            )

            idx_t = const.tile([128, idx_cols], dt.int16)
            nc.sync.dma_start(idx_t[:], idx_d.ap()[:, :])
            qt_t = const.tile([D, RPC * H], dt.bfloat16)
            nc.sync.dma_start(qt_t[:], qt_d.ap()[:, :])
            mask_t = const.tile([128, nch * G], dt.bfloat16)
            nc.sync.dma_start(mask_t[:], mask_d.ap()[:, :])
            ones_t = const.tile([128, 1], dt.float32)
            nc.vector.memset(ones_t[:], 1.0)

            colsum_t = const.tile([128, RPC * HKV * G], dt.float32)
            kt_t = const.tile([128, HKV * n_idx], dt.bfloat16)
            v_t = const.tile([128, HKV * n_idx], dt.bfloat16)
            expp_t = const.tile([128, nch * HKV * G], dt.bfloat16)
            out_t = const.tile([G, RPC * HKV * D], dt.float32)
            rden_t = const.tile([G, RPC * HKV], dt.float32)

            o_acc = accp.tile([G, RPC * HKV * D], dt.float32)
            den = accp.tile([G, RPC * HKV], dt.float32)

            kg_sems = [
                nc.alloc_semaphore(f"kg{i}")
                for i in range((n_idx + PERK - 1) // PERK)
            ]
            vg_sems = [
                nc.alloc_semaphore(f"vg{i}")
                for i in range((n_idx + PERV - 1) // PERV)
            ]

            issue = []
            for g0 in range(0, n_idx, PERK):
                issue.append(("k", g0))
            for g0 in range(0, n_idx, PERV):
                issue.append(("v", g0))
            # interleave by data position so K and V of the same chunks
            # arrive together; alternate the two SWDGE queues
            issue.sort(key=lambda t: (t[1], t[0]))
            for kind, g0 in issue:
                if kind == "k":
                    ng = min(PERK, n_idx - g0)
                    nc.gpsimd.dma_gather(
                        kt_t[:, g0 * HKV : (g0 + ng) * HKV].rearrange(
                            "p (b e) -> p b e", b=HKV
                        ),
                        kpool.ap()[:, :],
                        idx_t[:, g0 // 16 : (g0 + ng) // 16],
                        ng,
                        ng,
                        ROW,
                        transpose=True,
                        single_packet=False,
                        queue_num=0,
                    ).then_inc(kg_sems[g0 // PERK], 16)
                else:
                    ng = min(PERV, n_idx - g0)
                    nc.gpsimd.dma_gather(
                        v_t[:, g0 * HKV : (g0 + ng) * HKV].rearrange(
                            "p (b e) -> p b e", e=ROW
                        ),
                        vpool.ap()[:, :],
                        idx_t[:, g0 // 16 : (g0 + ng) // 16],
                        ng,
                        ng,
                        ROW,
                        single_packet=False,
                        queue_num=1,
                    ).then_inc(vg_sems[g0 // PERV], 16)

            def kt_slice(c, h):
                g, loc = divmod(c * CH, PERK)
                base = g * HKV * PERK + h * PERK + loc
                return kt_t[:, base : base + CH]

            # head-major: each (r, h) PSUM accumulation group completes
            # before the next starts (start=True clears the whole bank's
            # has_written bits, so groups sharing a bank must not interleave)
            for r in range(RPC):
                nch_l = nch_r[r]
                c0 = cum[r]
                for h in range(HKV):
                    blk = r * HKV + h
                    st = stp.tile([128, nch_r[0] * G], dt.float32, tag="st")
                    base = (c0 * HKV + h * nch_l) * G
                    for cl in range(nch_l):
                        nc.tensor.matmul(
                            st[:, cl * G : (cl + 1) * G],
                            kt_slice(c0 + cl, h),
                            qt_t[:, blk * G : (blk + 1) * G],
                            start=True,
                            stop=True,
                        )._wait_ge(kg_sems[(c0 + cl) * CH // PERK], 16)
                    pt = expp_t[:, base : base + nch_l * G]
                    nc.scalar.activation(
                        pt,
                        st[:, 0 : nch_l * G],
                        mybir.ActivationFunctionType.Exp,
                    )
                    nc.vector.tensor_mul(
                        pt, pt, mask_t[:, c0 * G : (c0 + nch_l) * G]
                    )
                    cs = colsum_t[:, blk * G : (blk + 1) * G]
                    nc.vector.tensor_reduce(
                        cs,
                        pt.rearrange("p (c g) -> p g c", g=G),
                        mybir.AxisListType.X,
                        mybir.AluOpType.add,
                    )
                    nc.tensor.matmul(
                        den[:, blk : blk + 1],
                        cs,
                        ones_t[:],
                        start=True,
                        stop=True,
                        skip_group_check=True,
                    )
                    for cl in range(nch_l):
                        c = c0 + cl
                        ptc = expp_t[:, base + cl * G : base + (cl + 1) * G]
                        nc.tensor.matmul(
                            o_acc[:, blk * D : (blk + 1) * D],
                            ptc,
                            v_t[:, (c * HKV + h) * D : (c * HKV + h + 1) * D],
                            start=(cl == 0),
                            stop=(cl == nch_l - 1),
                            skip_group_check=True,
                        )._wait_ge(vg_sems[c * CH // PERV], 16)
            # numerator/denominator go to host; division (flash-decode
            # stage-2 combine) happens there on the tiny output tensor
            nc.scalar.copy(out_t[:], o_acc[:])
            nc.vector.tensor_copy(rden_t[:], den[:])
            nc.sync.dma_start(out_d.ap()[:, :], out_t[:])
            nc.sync.dma_start(deno_d.ap()[:, :], rden_t[:])

    nc.compile()
    return nc


def kernel(q, k, v, k_cache, v_cache, slot_mapping, active_slots, context_lens):
    q = np.asarray(q)
    k = np.asarray(k)
    v = np.asarray(v)
    k_cache = np.asarray(k_cache)
    v_cache = np.asarray(v_cache)
    slot_mapping = np.asarray(slot_mapping)
    active_slots = np.asarray(active_slots)
    context_lens = np.asarray(context_lens).astype(np.int64)

    order = np.argsort(-context_lens, kind="stable")
    pairs = [(int(order[i]), int(order[B - 1 - i])) for i in range(NCORES)]

    nch_r = tuple(
        max(int(-(-context_lens[p[s]] // CH)) for p in pairs) for s in range(RPC)
    )
    nch = sum(nch_r)
    n_idx = nch * CH

    kc_new = k.astype(BF16)
    vc_new = v.astype(BF16)
    sm_ok = {}
    for i in range(B):
        s = int(slot_mapping[i])
        if 0 <= s < NUM_SLOTS:
            sm_ok[s] = i

    per_core = []
    for core in range(NCORES):
        rA, rB = pairs[core]
        flat = np.zeros(n_idx, np.int64)
        mask = np.zeros((128, nch * G), BF16)
        for s, r in enumerate((rA, rB)):
            L = int(context_lens[r])
            off = 0 if s == 0 else nch_r[0]
            flat[off * CH : off * CH + L] = active_slots[r, :L]
            for c in range(nch_r[s]):
                nv = min(max(L - c * CH, 0), CH)
                if nv > 0:
                    mask[:nv, (off + c) * G : (off + c + 1) * G] = 1.0
        uniq, inv = np.unique(flat, return_inverse=True)
        npool = len(uniq)
        assert npool < 32768
        kp = k_cache[uniq].astype(BF16).reshape(npool, ROW)
        vp = v_cache[uniq].astype(BF16).reshape(npool, ROW)
        for pos, s in enumerate(uniq):
            i = sm_ok.get(int(s))
            if i is not None:
                kp[pos] = kc_new[i].reshape(ROW)
                vp[pos] = vc_new[i].reshape(ROW)
        idx16 = inv.astype(np.int16)
        idx_w = np.tile(idx16.reshape(n_idx // 16, 16).T, (8, 1))

        qs = (q[(rA, rB), :, :] * SCALE).astype(BF16)
        qt = np.ascontiguousarray(qs.transpose(2, 0, 1).reshape(D, RPC * H))
        per_core.append(
            {"kp": kp, "vp": vp, "qt": qt, "idx": idx_w, "mask": mask}
        )

    npool_max = max(pc["kp"].shape[0] for pc in per_core)
    in_maps = []
    for pc in per_core:
        kp, vp = pc["kp"], pc["vp"]
        if kp.shape[0] < npool_max:
            pad = np.zeros((npool_max - kp.shape[0], ROW), BF16)
            kp = np.concatenate([kp, pad])
            vp = np.concatenate([vp, pad])
        in_maps.append(
            {
                "kpool": kp,
                "vpool": vp,
                "qt": pc["qt"],
                "idx": pc["idx"],
                "mask": pc["mask"],
            }
        )

    idx_cols = n_idx // 16
    key = (npool_max, nch_r, idx_cols)
    if key not in _compiled:
        _compiled[key] = _build(npool_max, nch_r, idx_cols)
    nc = _compiled[key]

    res = bass_utils.run_bass_kernel_spmd(nc, in_maps, core_ids=list(range(NCORES)))

    out = np.empty((B, H, D), np.float32)
    for core in range(NCORES):
        num = res.results[core]["o"].reshape(G, RPC, HKV, D)
        dn = res.results[core]["deno"].reshape(G, RPC, HKV, 1)
        o = num / dn
        for s, r in enumerate(pairs[core]):
            out[r] = o[:, s, :, :].transpose(1, 0, 2).reshape(H, D)
    return out
